# revision 5
# baseline (speedup 1.0000x reference)
"""Trainium2 Bass kernel for nn_AggCF_Module (block-local mixed-channel attn).

The reference's reshape (BS,H,W,NH*DK) -> (BS*NH, H*W, DK) is a PLAIN reshape:
attention batch = (image b, spatial block s of 8 rows / 512 tokens), the
"sequence" n = (token-in-block t, channel-group g), feature d = c%64.
So attention is local to each 512-token block; each query (t,g) attends over
all (u ptoken-in-block, g' channel-group) = 128*8 = 1024 keys of its block.

Sharding: 8 cores = (image b, half h). Core handles blocks s in [4h, 4h+4).
Inputs per core: x_loc [2112, 512] = the half's 2048 tokens + 64 boundary
tokens (zero-padded for h=1) + pooling edge-count fix scalar.

Device pipeline (bf16 matmuls, fp32 PSUM/residual):
  xT   = transpose(x_loc)                 DRAM bf16 staging + DMA-transpose
  xpT  = 3x3/s2 avgpool(xT)               DVE strided adds (free-dim shifts)
  QT   = w_q^T @ xT   [c, tok 2048]
  KT   = w_k^T @ xpT  [c, ptok 512];  KT_dup = parity-swapped copy (PE ident)
  V    = xpT^T @ w_v + ones col           [ptok-tile, g'*65+d]
  per block s, g-pair, g'-pair: S^T = K' Q'^T (row-paired K=64 matmuls)
  St = exp(S^T/8) (ACT, bf16);  O'[g] += [V|1]^T @ St  (M=65, denom in row 64)
  O = O'[0:64] * recip(O'[64]) broadcast; Z = O^T w_o + b_o + x (fp32)
"""

import numpy as np

N_CORES = 8
BS, H, W, C = 4, 64, 64, 512
NG, DK = 8, 64          # channel groups ("heads"), feature dim
T = H * W               # 4096 tokens per image
TQ = T // 2             # 2048 query tokens per core
TLOC = TQ + W           # 2112 = +1 boundary spatial row for pooling
HLOC = TLOC // W        # 33 input rows
PHL, PW = 16, W // 2    # local pooled rows, pooled cols
PML = PHL * PW          # 512 local pooled tokens
NBLK = 4                # spatial blocks per core (128 ptok / 512 tok each)
TEMP = 8.0
NPAIR = 4               # group pairs per 128 partitions

_CACHE = {}


def _build():
    import concourse.bass as bass
    import concourse.tile as tile
    import concourse.mybir as mybir
    from concourse import bacc
    from concourse.masks import make_identity

    F32 = mybir.dt.float32
    BF16 = mybir.dt.bfloat16
    Exp = mybir.ActivationFunctionType.Exp

    nc = bacc.Bacc("TRN2", target_bir_lowering=False, debug=False,
                   num_devices=N_CORES)

    x_loc = nc.dram_tensor("x_loc", [TLOC, C], F32, kind="ExternalInput")
    fix = nc.dram_tensor("fix", [1], F32, kind="ExternalInput")
    w_q = nc.dram_tensor("w_q", [C, C], F32, kind="ExternalInput")
    w_k = nc.dram_tensor("w_k", [C, C], F32, kind="ExternalInput")
    w_v = nc.dram_tensor("w_v", [C, C], F32, kind="ExternalInput")
    w_o = nc.dram_tensor("w_o", [C, C], F32, kind="ExternalInput")
    b_q = nc.dram_tensor("b_q", [C], F32, kind="ExternalInput")
    b_k = nc.dram_tensor("b_k", [C], F32, kind="ExternalInput")
    b_v = nc.dram_tensor("b_v", [C], F32, kind="ExternalInput")
    b_o = nc.dram_tensor("b_o", [C], F32, kind="ExternalInput")
    out = nc.dram_tensor("out", [TQ, C], F32, kind="ExternalOutput")

    with tile.TileContext(nc) as tc:
        with (
            tc.tile_pool(name="const", bufs=1) as const,
            tc.tile_pool(name="big", bufs=1) as big,
            tc.tile_pool(name="work", bufs=2) as work,
            tc.tile_pool(name="stp", bufs=4) as stp,
            tc.tile_pool(name="otp", bufs=2) as otp,
            tc.tile_pool(name="iop", bufs=3) as iop,
            tc.tile_pool(name="ps_s", bufs=2, space="PSUM") as ps_s,
            tc.tile_pool(name="ps_o", bufs=2, space="PSUM") as ps_o,
            tc.tile_pool(name="ps_m", bufs=2, space="PSUM") as ps_m,
            tc.tile_pool(name="dram", bufs=1, space="DRAM") as dram,
        ):
            # ---- constants ----
            wq_sb = const.tile([128, 4, C], BF16, tag="wq")
            wk_sb = const.tile([128, 4, C], BF16, tag="wk")
            wv_sb = const.tile([128, 4, C], BF16, tag="wv")
            for wt, wsb in ((w_q, wq_sb), (w_k, wk_sb), (w_v, wv_sb)):
                nc.gpsimd.dma_start(wsb[:], wt.rearrange("(k p) n -> p k n", p=128))
            wo_sb = const.tile([64, 8, C], BF16, tag="wo")
            nc.gpsimd.dma_start(wo_sb[:], w_o.rearrange("(h d) n -> d h n", d=64))

            bq_sb = const.tile([128, 4], F32, tag="bq")
            nc.sync.dma_start(bq_sb[:], b_q.rearrange("(a p) -> p a", p=128))
            bk_sb = const.tile([128, 4], F32, tag="bk")
            nc.sync.dma_start(bk_sb[:], b_k.rearrange("(a p) -> p a", p=128))
            bv_row = const.tile([1, C], BF16, tag="bv")
            nc.gpsimd.dma_start(bv_row[:], b_v[None, :])
            bo_row = const.tile([1, C], BF16, tag="bo")
            nc.gpsimd.dma_start(bo_row[:], b_o[None, :])
            ones_sb = const.tile([128, 128], BF16, tag="ones")
            nc.vector.memset(ones_sb[:], 1.0)
            fix_sb = const.tile([128, 1], F32, tag="fix")
            nc.gpsimd.dma_start(fix_sb[:], fix[:].to_broadcast([128, 1]))
            # identity [128, 64]: I2[p, m] = (p % 64 == m), for parity swaps
            i2 = const.tile([128, 64], BF16, tag="i2")
            nc.gpsimd.memset(i2[:], 0.0)
            make_identity(nc, i2[0:64, :], nomemset=True)
            make_identity(nc, i2[64:128, :], nomemset=True)

            # ---- stage x to DRAM bf16, transpose-load ----
            x_bf = dram.tile([TLOC, C], BF16, tag="xbf")
            nc.gpsimd.dma_start(x_bf[:], x_loc[:])
            xT = big.tile([128, 4, TLOC], BF16, tag="xT")
            for ct in range(4):
                nc.sync.dma_start_transpose(
                    xT[:, ct, :], x_bf[:, ct * 128:(ct + 1) * 128])

            # ---- Q^T projection over the 2048 query tokens ----
            QT = big.tile([128, NPAIR, TQ], BF16, tag="QT")
            for pair in range(NPAIR):
                for qc in range(4):
                    ps = ps_m.tile([128, 512], F32, tag="proj")
                    for kt in range(4):
                        nc.tensor.matmul(
                            ps[:],
                            wq_sb[:, kt, pair * 128:(pair + 1) * 128],
                            xT[:, kt, qc * 512:(qc + 1) * 512],
                            start=(kt == 0), stop=(kt == 3))
                    nc.vector.tensor_scalar_add(
                        QT[:, pair, qc * 512:(qc + 1) * 512],
                        ps[:], bq_sb[:, pair:pair + 1])

            # ---- pooling: xpT[c, (pi,pj)] over 16x32 local pooled grid ----
            xpT = big.tile([128, 4, PML], BF16, tag="xpT")
            for ct in range(4):
                xr = xT[:, ct, :].rearrange("p (i j) -> p i j", j=W)
                vs = work.tile([128, PHL, W], BF16, tag="vs")
                nc.vector.tensor_add(vs[:], xr[:, 0:HLOC - 2:2, :],
                                     xr[:, 1:HLOC - 1:2, :])
                nc.vector.tensor_add(vs[:], vs[:], xr[:, 2:HLOC:2, :])
                pr = xpT[:, ct, :].rearrange("p (i j) -> p i j", j=PW)
                nc.vector.tensor_add(pr[:, :, 0:PW - 1], vs[:, :, 0:W - 2:2],
                                     vs[:, :, 1:W - 1:2])
                nc.vector.tensor_add(pr[:, :, 0:PW - 1], pr[:, :, 0:PW - 1],
                                     vs[:, :, 2:W:2])
                nc.vector.tensor_add(pr[:, :, PW - 1], vs[:, :, W - 2],
                                     vs[:, :, W - 1])
                nc.vector.tensor_scalar_mul(xpT[:, ct, :], xpT[:, ct, :],
                                            1.0 / 9.0)
                # W-edge (pj=31): only 2 valid cols -> *1.5
                nc.vector.tensor_scalar_mul(pr[:, :, PW - 1], pr[:, :, PW - 1],
                                            1.5)
                # bottom edge (local pi=15): count per core half (1.0 or 1.5)
                nc.vector.tensor_scalar_mul(pr[:, PHL - 1, :],
                                            pr[:, PHL - 1, :],
                                            fix_sb[:, 0:1])

            # ---- K^T projection + parity-swapped duplicate ----
            KT = big.tile([128, NPAIR, PML], BF16, tag="KT")
            for pair in range(NPAIR):
                ps = ps_m.tile([128, 512], F32, tag="proj")
                for kt in range(4):
                    nc.tensor.matmul(
                        ps[:], wk_sb[:, kt, pair * 128:(pair + 1) * 128],
                        xpT[:, kt, :], start=(kt == 0), stop=(kt == 3))
                nc.vector.tensor_scalar_add(KT[:, pair, :], ps[:],
                                            bk_sb[:, pair:pair + 1])
            KTd = big.tile([128, NPAIR, PML], BF16, tag="KTd")
            for pair in range(NPAIR):
                ps = ps_m.tile([128, 512], F32, tag="proj")
                nc.tensor.matmul(ps[0:64, :], i2[64:128, :], KT[64:128, pair, :],
                                 start=True, stop=True, tile_position=(64, 0),
                                 skip_group_check=True)
                nc.tensor.matmul(ps[64:128, :], i2[0:64, :], KT[0:64, pair, :],
                                 start=True, stop=True, tile_position=(0, 64),
                                 skip_group_check=True)
                nc.vector.tensor_copy(KTd[:, pair, :], ps[:])

            # ---- V projection (+ ones col per group) ----
            V = big.tile([128, NBLK, 8 * 65], BF16, tag="V")
            for tt in range(NBLK):
                for half in range(2):
                    ps = ps_m.tile([128, 256], F32, tag="proj")
                    for kt in range(4):
                        nc.tensor.matmul(
                            ps[:], xpT[:, kt, tt * 128:(tt + 1) * 128],
                            wv_sb[:, kt, half * 256:(half + 1) * 256],
                            start=(kt == 0), stop=False)
                    nc.tensor.matmul(
                        ps[:], ones_sb[0:1, 0:128],
                        bv_row[:, half * 256:(half + 1) * 256],
                        start=False, stop=True)
                    nc.vector.tensor_copy(
                        V[:, tt, half * 260:half * 260 + 260]
                        .rearrange("p (h u) -> p h u", u=65)[:, :, 0:64],
                        ps[:].rearrange("p (h u) -> p h u", u=64))
                one_cols = V[:, tt, :].rearrange("p (h u) -> p h u", u=65)[:, :, 64]
                nc.vector.memset(one_cols, 1.0)

            # ---- attention per block s ----
            for s in range(NBLK):
                OT = otp.tile([64, 8, 512], BF16, tag="OT")
                for pair in range(NPAIR):
                    o_ps = []
                    for _h in range(2):
                        o_tile = ps_o.tile([128, 512], F32, tag="o",
                                           name=f"o_{s}_{pair}_{_h}")
                        o_ps.append(o_tile)
                    for qp in range(4):   # g'-pair index
                        for y in range(2):   # X: parity-matched, Y: crossed
                            s_ps = ps_s.tile([128, 2, 512], F32, tag="s")
                            for h2 in range(2):
                                kt_src = KT if y == 0 else KTd
                                # lhsT rows (h2*64..): group g' = 2qp + (h2^y)
                                nc.tensor.matmul(
                                    s_ps[:, h2, :],
                                    kt_src[h2 * 64:(h2 + 1) * 64, qp,
                                           s * 128:(s + 1) * 128],
                                    QT[h2 * 64:(h2 + 1) * 64, pair,
                                       s * 512:(s + 1) * 512],
                                    start=True, stop=True,
                                    tile_position=(h2 * 64, 0))
                            st = stp.tile([128, 2, 512], BF16, tag="st")
                            nc.scalar.activation(
                                st[:].rearrange("p a b -> p (a b)"),
                                s_ps[:].rearrange("p a b -> p (a b)"),
                                Exp, scale=1.0 / TEMP)
                            for h2 in range(2):
                                gp = 2 * qp + (h2 ^ y)   # key group
                                nc.tensor.matmul(
                                    o_ps[h2][0:65, :],
                                    V[:, s, gp * 65:(gp + 1) * 65],
                                    st[:, h2, :],
                                    start=(qp == 0 and y == 0),
                                    stop=(qp == 3 and y == 1),
                                    skip_group_check=True)
                    for h2 in range(2):
                        g = 2 * pair + h2
                        rcp = work.tile([128, 512], BF16, tag="rcp")
                        with nc.allow_low_precision(
                                reason="softmax denom recip in bf16"):
                            nc.vector.reciprocal(rcp[64:65, :],
                                                 o_ps[h2][64:65, :])
                        bc_ps = ps_m.tile([64, 512], F32, tag="proj")
                        nc.tensor.matmul(bc_ps[:], ones_sb[64:65, 0:64],
                                         rcp[64:65, :], start=True, stop=True,
                                         tile_position=(64, 0))
                        bc_sb = work.tile([64, 512], BF16, tag="bc")
                        nc.vector.tensor_copy(bc_sb[:], bc_ps[:])
                        nc.vector.tensor_mul(OT[:, g, :], o_ps[h2][0:64, :],
                                             bc_sb[:])
                # ---- Z = OT^T @ w_o + b_o + residual ----
                for tt in range(4):
                    tok0 = s * 512 + tt * 128
                    z_ps = ps_m.tile([128, C], F32, tag="proj")
                    for g in range(8):
                        nc.tensor.matmul(
                            z_ps[:], OT[:, g, tt * 128:(tt + 1) * 128],
                            wo_sb[:, g, :], start=(g == 0), stop=False)
                    nc.tensor.matmul(z_ps[:], ones_sb[0:1, 0:128], bo_row[:],
                                     start=False, stop=True)
                    xr_t = iop.tile([128, C], F32, tag="xr")
                    nc.sync.dma_start(xr_t[:], x_loc[tok0:tok0 + 128, :])
                    o_sb = iop.tile([128, C], F32, tag="osb")
                    nc.vector.tensor_add(o_sb[:], z_ps[:], xr_t[:])
                    nc.sync.dma_start(out[tok0:tok0 + 128, :], o_sb[:])

    nc.compile()
    return nc


def _get_nc():
    if "nc" not in _CACHE:
        _CACHE["nc"] = _build()
    return _CACHE["nc"]


def kernel(**inputs):
    from concourse.bass_utils import run_bass_kernel_spmd

    x = np.ascontiguousarray(np.asarray(inputs["x"], dtype=np.float32))
    ws = {k: np.ascontiguousarray(np.asarray(inputs[k], dtype=np.float32))
          for k in ("w_q", "w_k", "w_v", "w_o", "b_q", "b_k", "b_v", "b_o")}

    nc = _get_nc()
    in_maps = []
    for c in range(N_CORES):
        img, half = c // 2, c % 2
        x_img = x[img].reshape(T, C)
        x_l = np.zeros((TLOC, C), np.float32)
        end = min((half * TQ) + TLOC, T)
        x_l[:end - half * TQ] = x_img[half * TQ:end]
        m = {"x_loc": x_l,
             "fix": np.array([1.0 if half == 0 else 1.5], np.float32)}
        m.update(ws)
        in_maps.append(m)

    res = run_bass_kernel_spmd(nc, in_maps, core_ids=list(range(N_CORES)),
                               **_CACHE.get("run_kwargs", {}))
    _CACHE["last_result"] = res

    outp = np.empty((BS, T, C), dtype=np.float32)
    for c in range(N_CORES):
        img, half = c // 2, c % 2
        outp[img, half * TQ:(half + 1) * TQ] = res.results[c]["out"]
    return outp.reshape(BS, H, W, C)


# revision 7
# speedup vs baseline: 1.0963x; 1.0963x over previous
"""Trainium2 Bass kernel for nn_AggCF_Module (block-local mixed-channel attn).

The reference's reshape (BS,H,W,NH*DK) -> (BS*NH, H*W, DK) is a PLAIN reshape:
attention batch = (image b, spatial block s of 8 rows / 512 tokens), the
"sequence" n = (token-in-block t, channel-group g), feature d = c%64.
So attention is local to each 512-token block; each query (t,g) attends over
all (u ptoken-in-block, g' channel-group) = 128*8 = 1024 keys of its block.

Sharding: 8 cores = (image b, half h). Core handles blocks s in [4h, 4h+4).
Inputs per core: x_loc [2112, 512] = the half's 2048 tokens + 64 boundary
tokens (zero-padded for h=1) + pooling edge-count fix scalar.

Device pipeline (bf16 matmuls, fp32 PSUM/residual):
  xT   = transpose(x_loc)                 DRAM bf16 staging + DMA-transpose
  xpT  = 3x3/s2 avgpool(xT)               DVE strided adds (free-dim shifts)
  QT   = w_q^T @ xT   [c, tok 2048]
  KT   = w_k^T @ xpT  [c, ptok 512];  KT_dup = parity-swapped copy (PE ident)
  V    = xpT^T @ w_v + ones col           [ptok-tile, g'*65+d]
  per block s, g-pair, g'-pair: S^T = K' Q'^T (row-paired K=64 matmuls)
  St = exp(S^T/8) (ACT, bf16);  O'[g] += [V|1]^T @ St  (M=65, denom in row 64)
  O = O'[0:64] * recip(O'[64]) broadcast; Z = O^T w_o + b_o + x (fp32)
"""

import numpy as np

N_CORES = 8
BS, H, W, C = 4, 64, 64, 512
NG, DK = 8, 64          # channel groups ("heads"), feature dim
T = H * W               # 4096 tokens per image
TQ = T // 2             # 2048 query tokens per core
TLOC = TQ + W           # 2112 = +1 boundary spatial row for pooling
HLOC = TLOC // W        # 33 input rows
PHL, PW = 16, W // 2    # local pooled rows, pooled cols
PML = PHL * PW          # 512 local pooled tokens
NBLK = 4                # spatial blocks per core (128 ptok / 512 tok each)
TEMP = 8.0
NPAIR = 4               # group pairs per 128 partitions

_CACHE = {}


def _build():
    import concourse.bass as bass
    import concourse.tile as tile
    import concourse.mybir as mybir
    from concourse import bacc
    from concourse.masks import make_identity

    F32 = mybir.dt.float32
    BF16 = mybir.dt.bfloat16
    Exp = mybir.ActivationFunctionType.Exp

    nc = bacc.Bacc("TRN2", target_bir_lowering=False, debug=False,
                   num_devices=N_CORES)

    x_loc = nc.dram_tensor("x_loc", [TLOC, C], F32, kind="ExternalInput")
    fix = nc.dram_tensor("fix", [1], F32, kind="ExternalInput")
    w_q = nc.dram_tensor("w_q", [C, C], F32, kind="ExternalInput")
    w_k = nc.dram_tensor("w_k", [C, C], F32, kind="ExternalInput")
    w_v = nc.dram_tensor("w_v", [C, C], F32, kind="ExternalInput")
    w_o = nc.dram_tensor("w_o", [C, C], F32, kind="ExternalInput")
    b_q = nc.dram_tensor("b_q", [C], F32, kind="ExternalInput")
    b_k = nc.dram_tensor("b_k", [C], F32, kind="ExternalInput")
    b_v = nc.dram_tensor("b_v", [C], F32, kind="ExternalInput")
    b_o = nc.dram_tensor("b_o", [C], F32, kind="ExternalInput")
    out = nc.dram_tensor("out", [TQ, C], F32, kind="ExternalOutput")

    with tile.TileContext(nc) as tc:
        with (
            tc.tile_pool(name="const", bufs=1) as const,
            tc.tile_pool(name="big", bufs=1) as big,
            tc.tile_pool(name="work", bufs=2) as work,
            tc.tile_pool(name="stp", bufs=6) as stp,
            tc.tile_pool(name="otp", bufs=2) as otp,
            tc.tile_pool(name="iop", bufs=3) as iop,
            tc.tile_pool(name="ps_s", bufs=2, space="PSUM") as ps_s,
            tc.tile_pool(name="ps_o", bufs=2, space="PSUM") as ps_o,
            tc.tile_pool(name="dram", bufs=1, space="DRAM") as dram,
        ):
            # ---- constants ----
            wq_sb = const.tile([128, 4, C], BF16, tag="wq")
            wk_sb = const.tile([128, 4, C], BF16, tag="wk")
            wv_sb = const.tile([128, 4, C], BF16, tag="wv")
            for wt, wsb in ((w_q, wq_sb), (w_k, wk_sb), (w_v, wv_sb)):
                nc.gpsimd.dma_start(wsb[:], wt.rearrange("(k p) n -> p k n", p=128))
            wo_sb = const.tile([64, 8, C], BF16, tag="wo")
            nc.gpsimd.dma_start(wo_sb[:], w_o.rearrange("(h d) n -> d h n", d=64))

            bq_sb = const.tile([128, 4], F32, tag="bq")
            nc.sync.dma_start(bq_sb[:], b_q.rearrange("(a p) -> p a", p=128))
            bk_sb = const.tile([128, 4], F32, tag="bk")
            nc.sync.dma_start(bk_sb[:], b_k.rearrange("(a p) -> p a", p=128))
            bv_row = const.tile([1, C], BF16, tag="bv")
            nc.gpsimd.dma_start(bv_row[:], b_v[None, :])
            bo_row = const.tile([1, C], BF16, tag="bo")
            nc.gpsimd.dma_start(bo_row[:], b_o[None, :])
            ones_sb = const.tile([128, 128], BF16, tag="ones")
            nc.vector.memset(ones_sb[:], 1.0)
            fix_sb = const.tile([128, 1], F32, tag="fix")
            nc.gpsimd.dma_start(fix_sb[:], fix[:].to_broadcast([128, 1]))
            # identity [128, 64]: I2[p, m] = (p % 64 == m), for parity swaps
            i2 = const.tile([128, 64], BF16, tag="i2")
            nc.gpsimd.memset(i2[:], 0.0)
            make_identity(nc, i2[0:64, :], nomemset=True)
            make_identity(nc, i2[64:128, :], nomemset=True)

            # ---- stage x to DRAM bf16, transpose-load ----
            x_bf = dram.tile([TLOC, C], BF16, tag="xbf")
            RC = TLOC // 4  # 528-row chunks
            for rc in range(4):
                nc.gpsimd.dma_start(x_bf[rc * RC:(rc + 1) * RC, :],
                                    x_loc[rc * RC:(rc + 1) * RC, :])
            xT = big.tile([128, 4, TLOC], BF16, tag="xT")
            for rc in range(4):
                for ct in range(4):
                    nc.sync.dma_start_transpose(
                        xT[:, ct, rc * RC:(rc + 1) * RC],
                        x_bf[rc * RC:(rc + 1) * RC, ct * 128:(ct + 1) * 128])

            # ---- Q^T projection over the 2048 query tokens ----
            QT = big.tile([128, NPAIR, TQ], BF16, tag="QT")
            for pair in range(NPAIR):
                for qc in range(4):
                    ps = ps_s.tile([128, 512], F32, tag="s")
                    for kt in range(4):
                        nc.tensor.matmul(
                            ps[:],
                            wq_sb[:, kt, pair * 128:(pair + 1) * 128],
                            xT[:, kt, qc * 512:(qc + 1) * 512],
                            start=(kt == 0), stop=(kt == 3))
                    nc.vector.tensor_scalar_add(
                        QT[:, pair, qc * 512:(qc + 1) * 512],
                        ps[:], bq_sb[:, pair:pair + 1])

            # ---- pooling: xpT[c, (pi,pj)] over 16x32 local pooled grid ----
            xpT = big.tile([128, 4, PML], BF16, tag="xpT")
            for ct in range(4):
                xr = xT[:, ct, :].rearrange("p (i j) -> p i j", j=W)
                vs = work.tile([128, PHL, W], BF16, tag="vs")
                nc.vector.tensor_add(vs[:], xr[:, 0:HLOC - 2:2, :],
                                     xr[:, 1:HLOC - 1:2, :])
                nc.vector.tensor_add(vs[:], vs[:], xr[:, 2:HLOC:2, :])
                pr = xpT[:, ct, :].rearrange("p (i j) -> p i j", j=PW)
                nc.vector.tensor_add(pr[:, :, 0:PW - 1], vs[:, :, 0:W - 2:2],
                                     vs[:, :, 1:W - 1:2])
                nc.vector.tensor_add(pr[:, :, 0:PW - 1], pr[:, :, 0:PW - 1],
                                     vs[:, :, 2:W:2])
                nc.vector.tensor_add(pr[:, :, PW - 1], vs[:, :, W - 2],
                                     vs[:, :, W - 1])
                nc.vector.tensor_scalar_mul(xpT[:, ct, :], xpT[:, ct, :],
                                            1.0 / 9.0)
                # W-edge (pj=31): only 2 valid cols -> *1.5
                nc.vector.tensor_scalar_mul(pr[:, :, PW - 1], pr[:, :, PW - 1],
                                            1.5)
                # bottom edge (local pi=15): count per core half (1.0 or 1.5)
                nc.vector.tensor_scalar_mul(pr[:, PHL - 1, :],
                                            pr[:, PHL - 1, :],
                                            fix_sb[:, 0:1])

            # ---- K^T projection + parity-swapped duplicate ----
            KT = big.tile([128, NPAIR, PML], BF16, tag="KT")
            for pair in range(NPAIR):
                ps = ps_s.tile([128, 512], F32, tag="s")
                for kt in range(4):
                    nc.tensor.matmul(
                        ps[:], wk_sb[:, kt, pair * 128:(pair + 1) * 128],
                        xpT[:, kt, :], start=(kt == 0), stop=(kt == 3))
                nc.vector.tensor_scalar_add(KT[:, pair, :], ps[:],
                                            bk_sb[:, pair:pair + 1])
            KTd = big.tile([128, NPAIR, PML], BF16, tag="KTd")
            for pair in range(NPAIR):
                ps = ps_s.tile([128, 512], F32, tag="s")
                nc.tensor.matmul(ps[0:64, :], i2[64:128, :], KT[64:128, pair, :],
                                 start=True, stop=True, tile_position=(64, 0),
                                 skip_group_check=True)
                nc.tensor.matmul(ps[64:128, :], i2[0:64, :], KT[0:64, pair, :],
                                 start=True, stop=True, tile_position=(0, 64),
                                 skip_group_check=True)
                nc.vector.tensor_copy(KTd[:, pair, :], ps[:])

            # ---- V projection (+ ones col per group) ----
            V = big.tile([128, NBLK, 8 * 65], BF16, tag="V")
            for tt in range(NBLK):
                for half in range(2):
                    ps = ps_s.tile([128, 256], F32, tag="s")
                    for kt in range(4):
                        nc.tensor.matmul(
                            ps[:], xpT[:, kt, tt * 128:(tt + 1) * 128],
                            wv_sb[:, kt, half * 256:(half + 1) * 256],
                            start=(kt == 0), stop=False)
                    nc.tensor.matmul(
                        ps[:], ones_sb[0:1, 0:128],
                        bv_row[:, half * 256:(half + 1) * 256],
                        start=False, stop=True)
                    nc.vector.tensor_copy(
                        V[:, tt, half * 260:half * 260 + 260]
                        .rearrange("p (h u) -> p h u", u=65)[:, :, 0:64],
                        ps[:].rearrange("p (h u) -> p h u", u=64))
                one_cols = V[:, tt, :].rearrange("p (h u) -> p h u", u=65)[:, :, 64]
                nc.vector.memset(one_cols, 1.0)

            # ---- attention per block s ----
            for s in range(NBLK):
                OT = otp.tile([64, 8, 512], BF16, tag="OT")
                for pair in range(NPAIR):
                    o_pair = ps_o.tile([128, 2, 512], F32, tag="o",
                                       name=f"o_{s}_{pair}")
                    for qp in range(4):   # g'-pair index
                        for y in range(2):   # X: parity-matched, Y: crossed
                            s_ps = ps_s.tile([128, 2, 512], F32, tag="s")
                            for h2 in range(2):
                                kt_src = KT if y == 0 else KTd
                                # lhsT rows (h2*64..): group g' = 2qp + (h2^y)
                                nc.tensor.matmul(
                                    s_ps[:, h2, :],
                                    kt_src[h2 * 64:(h2 + 1) * 64, qp,
                                           s * 128:(s + 1) * 128],
                                    QT[h2 * 64:(h2 + 1) * 64, pair,
                                       s * 512:(s + 1) * 512],
                                    start=True, stop=True,
                                    tile_position=(h2 * 64, 0))
                            st = stp.tile([128, 2, 512], BF16, tag="st")
                            nc.scalar.activation(
                                st[:].rearrange("p a b -> p (a b)"),
                                s_ps[:].rearrange("p a b -> p (a b)"),
                                Exp, scale=1.0 / TEMP)
                            for h2 in range(2):
                                gp = 2 * qp + (h2 ^ y)   # key group
                                nc.tensor.matmul(
                                    o_pair[0:65, h2, :],
                                    V[:, s, gp * 65:(gp + 1) * 65],
                                    st[:, h2, :],
                                    start=(qp == 0 and y == 0),
                                    stop=(qp == 3 and y == 1),
                                    skip_group_check=True)
                    rc_bf = work.tile([128, 2, 512], BF16, tag="rcbf")
                    nc.scalar.copy(rc_bf[64:65, :, :], o_pair[64:65, :, :])
                    bc_ps = ps_s.tile([128, 2, 512], F32, tag="s")
                    for h2 in range(2):
                        nc.tensor.matmul(bc_ps[0:64, h2, :],
                                         ones_sb[64:65, 0:64],
                                         rc_bf[64:65, h2, :],
                                         start=True, stop=True,
                                         tile_position=(64, 0),
                                         skip_group_check=True)
                    rinv = work.tile([64, 2, 512], F32, tag="rinv")
                    nc.vector.reciprocal_approx_fast(rinv[:],
                                                     bc_ps[0:64, :, :])
                    for h2 in range(2):
                        g = 2 * pair + h2
                        nc.vector.tensor_mul(OT[:, g, :],
                                             o_pair[0:64, h2, :],
                                             rinv[:, h2, :])
                # ---- Z = OT^T @ w_o + b_o + residual ----
                for tt in range(4):
                    tok0 = s * 512 + tt * 128
                    z_ps = ps_s.tile([128, C], F32, tag="s")
                    for g in range(8):
                        nc.tensor.matmul(
                            z_ps[:], OT[:, g, tt * 128:(tt + 1) * 128],
                            wo_sb[:, g, :], start=(g == 0), stop=False)
                    nc.tensor.matmul(z_ps[:], ones_sb[0:1, 0:128], bo_row[:],
                                     start=False, stop=True)
                    xr_t = iop.tile([128, C], F32, tag="xr")
                    nc.sync.dma_start(xr_t[:], x_loc[tok0:tok0 + 128, :])
                    o_sb = iop.tile([128, C], F32, tag="osb")
                    nc.vector.tensor_add(o_sb[:], z_ps[:], xr_t[:])
                    nc.sync.dma_start(out[tok0:tok0 + 128, :], o_sb[:])

    nc.compile()
    return nc


def _get_nc():
    if "nc" not in _CACHE:
        _CACHE["nc"] = _build()
    return _CACHE["nc"]


def kernel(**inputs):
    from concourse.bass_utils import run_bass_kernel_spmd

    x = np.ascontiguousarray(np.asarray(inputs["x"], dtype=np.float32))
    ws = {k: np.ascontiguousarray(np.asarray(inputs[k], dtype=np.float32))
          for k in ("w_q", "w_k", "w_v", "w_o", "b_q", "b_k", "b_v", "b_o")}

    nc = _get_nc()
    in_maps = []
    for c in range(N_CORES):
        img, half = c // 2, c % 2
        x_img = x[img].reshape(T, C)
        x_l = np.zeros((TLOC, C), np.float32)
        end = min((half * TQ) + TLOC, T)
        x_l[:end - half * TQ] = x_img[half * TQ:end]
        m = {"x_loc": x_l,
             "fix": np.array([1.0 if half == 0 else 1.5], np.float32)}
        m.update(ws)
        in_maps.append(m)

    res = run_bass_kernel_spmd(nc, in_maps, core_ids=list(range(N_CORES)),
                               **_CACHE.get("run_kwargs", {}))
    _CACHE["last_result"] = res

    outp = np.empty((BS, T, C), dtype=np.float32)
    for c in range(N_CORES):
        img, half = c // 2, c % 2
        outp[img, half * TQ:(half + 1) * TQ] = res.results[c]["out"]
    return outp.reshape(BS, H, W, C)
